# revision 18
# baseline (speedup 1.0000x reference)
"""AM-Softmax loss (margin=0.3, scale=30, label smoothing 0.1) on 8 TRN2 cores.

Vocab-parallel: classifier weight (C,d) sharded along C across 8 cores.
Per core: L2-normalize weight shard + replicated features on device (rsqrt =
bit-trick seed + 2 Newton steps, all on VectorE), transpose what W^T via the
DMA xbar (DRAM round trip in bf16), then bf16 matmuls f_hat @ w_hat^T in
2048-column PSUM chunks fused with ScalarE exp(30*cos-30) + free-axis
accumulation into per-sample partial sum-exp Z.  cos <= 1 bounds logits by 30,
so a fixed offset replaces the running max (no all-reduce max).  Margin and
label-smoothing terms are exact analytic corrections:

  loss_i = lse_i - 0.9*(l_i - 9) - (0.1/C)*(T_i - 9)
  lse_i  = 30 + ln(Z_i + (e^-9 - 1) * e^(l_i - 30))
  T_i    = 30 * f_hat_i . (sum_c w_hat_c)     (one extra matmul column)
  l_i    = 30 * f_hat_i . w_hat_{y_i}         (indirect-DMA gather on owner)

One 24 KB AllReduce combines per-core partials (Z, T, l); every core computes
the final scalar; core 0's output is returned.
"""

import math

import numpy as np

import concourse.bass as bass
import concourse.bacc as bacc
import concourse.mybir as mybir
from concourse import tile

P = 128
B, D, C = 2048, 128, 100000
NCORES = 8
CS = C // NCORES
S, MARG, EPS = 30.0, 0.3, 0.1

f32 = mybir.dt.float32
bf16 = mybir.dt.bfloat16
i32 = mybir.dt.int32
FT = mybir.ActivationFunctionType
OP = mybir.AluOpType

MAGIC = 0x5F3759DF


def build_graph(num_cores=NCORES, b_sz=B, cs=CS, chunk=2048, wbatch=14):
    nb = b_sz // P  # B tiles
    nwt = math.ceil(cs / P)  # weight row tiles
    scol = nwt * P  # zero-padded row width; also the s-vector column
    nchunk = math.ceil(scol / chunk)  # class chunks per B-tile row
    nbatch = math.ceil(nwt / wbatch)  # DMA batches of weight tiles
    c_total = cs * num_cores
    ncw = max(nb, wbatch)  # norm-chain scratch width
    pe_tp_batches = min(3, nbatch)  # first batches transposed on TensorE

    kappa_m1 = float(np.exp(-S * MARG) - 1.0)
    const = float(S + (1.0 - EPS) * S * MARG + EPS * S * MARG / c_total)

    nc = bacc.Bacc(
        "TRN2", target_bir_lowering=False, debug=False, num_devices=num_cores
    )

    f_ext = nc.dram_tensor("f", [b_sz, D], f32, kind="ExternalInput")
    w_ext = nc.dram_tensor("w", [cs, D], f32, kind="ExternalInput")
    lab_ext = nc.dram_tensor("lab", [P, nb], i32, kind="ExternalInput")
    coff_ext = nc.dram_tensor("coff", [P, 1], f32, kind="ExternalInput")
    id_ext = nc.dram_tensor("id32", [P, P], f32, kind="ExternalInput")
    out_ext = nc.dram_tensor("out", [1, 1], f32, kind="ExternalOutput")

    with tile.TileContext(nc) as tc:
        with (
            tc.tile_pool(name="consts", bufs=1) as consts,
            tc.tile_pool(name="persist", bufs=1) as persist,
            tc.tile_pool(name="wa", bufs=3) as wap,
            tc.tile_pool(name="wsc", bufs=3) as wscp,
            tc.tile_pool(name="wlab", bufs=max(nb, 2)) as wlabp,
            tc.tile_pool(name="small", bufs=3) as smallp,
            tc.tile_pool(name="psum_mm", bufs=2, space="PSUM") as psmm,
            tc.tile_pool(name="dram", bufs=1, space="DRAM") as dramp,
        ):
            ident32 = consts.tile([P, P], f32, name="ident32")
            nc.scalar.dma_start(out=ident32[:], in_=id_ext[:, :])
            ident = consts.tile([P, P], bf16, name="ident")
            nc.vector.tensor_copy(out=ident[:], in_=ident32[:])
            bias_m30 = consts.tile([P, 1], f32, name="bias_m30")
            nc.vector.memset(bias_m30[:], -S)
            magic = consts.tile([P, ncw], i32, name="magic")
            nc.vector.memset(magic[:], MAGIC)
            onei = consts.tile([P, ncw], i32, name="onei")
            nc.vector.memset(onei[:], 1)

            def rsqrt(ssq_ap, inv_ap, n):
                """inv = 1/sqrt(ssq): quake seed + 2 Newton steps, DVE only."""
                half = smallp.tile([P, n], i32, tag="nrm_h", name="nrm_h")
                y0i = smallp.tile([P, n], i32, tag="nrm_yi", name="nrm_yi")
                t = smallp.tile([P, n], f32, tag="nrm_t", name="nrm_t")
                y1 = smallp.tile([P, n], f32, tag="nrm_y1", name="nrm_y1")
                nc.vector.tensor_tensor(
                    out=half[:], in0=ssq_ap.bitcast(i32), in1=onei[:, 0:n],
                    op=OP.arith_shift_right,
                )
                nc.vector.tensor_tensor(
                    out=y0i[:], in0=magic[:, 0:n], in1=half[:], op=OP.subtract
                )
                y0 = y0i[:].bitcast(f32)
                nc.vector.tensor_tensor(out=t[:], in0=y0, in1=y0, op=OP.mult)
                nc.vector.tensor_tensor(out=t[:], in0=t[:], in1=ssq_ap, op=OP.mult)
                nc.vector.tensor_scalar(
                    out=t[:], in0=t[:], scalar1=-0.5, scalar2=1.5,
                    op0=OP.mult, op1=OP.add,
                )
                nc.vector.tensor_tensor(out=y1[:], in0=y0, in1=t[:], op=OP.mult)
                nc.vector.tensor_tensor(out=t[:], in0=y1[:], in1=y1[:], op=OP.mult)
                nc.vector.tensor_tensor(out=t[:], in0=t[:], in1=ssq_ap, op=OP.mult)
                nc.vector.tensor_scalar(
                    out=t[:], in0=t[:], scalar1=-0.5, scalar2=1.5,
                    op0=OP.mult, op1=OP.add,
                )
                nc.vector.tensor_tensor(out=inv_ap, in0=y1[:], in1=t[:], op=OP.mult)

            # persistent SBUF state
            wT = persist.tile([P, scol + 8], bf16, name="wT")
            fa = persist.tile([P, nb * P], f32, name="fa")
            fnorm = persist.tile([P, nb * P], f32, name="fnorm")
            fT = persist.tile([P, nb * P], bf16, name="fT")
            ssqF = persist.tile([P, nb], f32, name="ssqF")
            invF = persist.tile([P, nb], f32, name="invF")
            ssqW = persist.tile([P, nwt], f32, name="ssqW")
            invW = persist.tile([P, nwt], f32, name="invW")
            ssqL = persist.tile([P, nb], f32, name="ssqL")
            invL = persist.tile([P, nb], f32, name="invL")
            ZP = persist.tile([P, nb * nchunk], f32, name="ZP")
            dots = persist.tile([P, nb], f32, name="dots")
            labi = persist.tile([P, nb], i32, name="labi")
            labf = persist.tile([P, nb], f32, name="labf")
            locc = persist.tile([P, nb], f32, name="locc")
            mask = persist.tile([P, nb], f32, name="mask")
            loci = persist.tile([P, nb], i32, name="loci")
            coff = persist.tile([P, 1], f32, name="coff")
            s32 = persist.tile([P, 1], f32, name="s32")
            ccin = persist.tile([P, 3 * nb], f32, name="ccin")
            R = persist.tile([P, 3 * nb], f32, name="R")
            wbf_d = dramp.tile([scol, D], bf16, name="wbf_d")
            if scol > cs:
                # zero-fill pad rows of the bf16 scratch (they transpose into
                # the zero pad columns of wT; also keeps the sim NaN-free)
                zpad = consts.tile([P, D], bf16, name="zpad")
                nc.vector.memset(zpad[:], 0.0)
                nc.gpsimd.dma_start(
                    out=wbf_d[cs:scol, :], in_=zpad[0 : scol - cs, :]
                )

            # ---- input DMAs -------------------------------------------------
            # f: sample (p*nb + b) -> partition p, col-block b
            nc.sync.dma_start(
                out=fa[:].rearrange("p (b d) -> p b d", d=D),
                in_=f_ext[:, :].rearrange("(p b) d -> p b d", b=nb),
            )
            nc.scalar.dma_start(out=labi[:], in_=lab_ext[:, :])
            nc.scalar.dma_start(out=coff[:], in_=coff_ext[:, :])

            # ---- f: ssq -> rsqrt -> scale -> transpose (PE, psum_mm pool) ---
            for b in range(nb):
                scr = smallp.tile([P, P], f32, tag="ttr_scr", name="ttr_scr")
                nc.vector.scalar_tensor_tensor(
                    out=scr[:], in0=fa[:, b * P : (b + 1) * P], scalar=1.0,
                    in1=fa[:, b * P : (b + 1) * P], op0=OP.mult, op1=OP.mult,
                    accum_out=ssqF[:, b : b + 1],
                )
            rsqrt(ssqF[:, 0:nb], invF[:, 0:nb], nb)
            for b in range(nb):
                sl = slice(b * P, (b + 1) * P)
                nc.vector.tensor_scalar(
                    out=fnorm[:, sl], in0=fa[:, sl], scalar1=invF[:, b : b + 1],
                    scalar2=None, op0=OP.mult,
                )
                fsc = wscp.tile([P, P], bf16, tag="fsc", name="fsc")
                nc.vector.tensor_copy(out=fsc[:], in_=fnorm[:, sl])
                tp = psmm.tile([P, P], bf16, tag="pm", name="tp_f")
                nc.tensor.transpose(out=tp[:], in_=fsc[:], identity=ident[:])
                nc.vector.tensor_copy(out=fT[:, sl], in_=tp[:])

            # ---- label localization + gathers + dots ------------------------
            nc.vector.tensor_copy(out=labf[:], in_=labi[:])
            nc.vector.tensor_scalar(
                out=locc[:], in0=labf[:], scalar1=coff[:, 0:1], scalar2=0.0,
                op0=OP.subtract, op1=OP.max,
            )
            nc.vector.tensor_scalar(
                out=locc[:], in0=locc[:], scalar1=float(cs - 1), scalar2=None,
                op0=OP.min,
            )
            scrm = smallp.tile([P, nb], f32, tag="scrm", name="scrm")
            nc.vector.tensor_scalar(
                out=scrm[:], in0=labf[:], scalar1=coff[:, 0:1], scalar2=None,
                op0=OP.subtract,
            )
            nc.vector.tensor_tensor(
                out=mask[:], in0=scrm[:], in1=locc[:], op=OP.is_equal
            )
            nc.vector.tensor_copy(out=loci[:], in_=locc[:])

            wlabs = []
            for b in range(nb):
                wlab = wlabp.tile([P, D], f32, tag="wlab", name=f"wlab{b}")
                wlabs.append(wlab)
                nc.gpsimd.indirect_dma_start(
                    out=wlab[:],
                    out_offset=None,
                    in_=w_ext[:, :],
                    in_offset=bass.IndirectOffsetOnAxis(ap=loci[:, b : b + 1], axis=0),
                )
            # ---- w pipeline (software-pipelined trace order) ----------------
            # stage A(k): DMA-in batch k (sync queue)
            # stage B(k-2): ssq -> rsqrt -> scale (DVE), DMA-out (gpsimd),
            #               xbar transpose-in (sync) -- so the sync queue sees
            #               [w0 w1 w2 wT0 w3 wT1 ...] and never idles.
            was = {}
            for k in range(nbatch + 2):
                if k < nbatch:
                    r0 = k * wbatch * P
                    rows = min(wbatch * P, cs - r0)
                    full_t = rows // P
                    rem = rows - full_t * P
                    nt = full_t + (1 if rem else 0)
                    wa = wap.tile([P, wbatch * P], f32, tag="wa", name=f"wa{k}")
                    was[k] = (wa, nt, r0, rows, full_t, rem)
                    if rem:
                        # pad rows: harmless nonzero so rsqrt stays finite
                        nc.vector.memset(wa[:, full_t * P : (full_t + 1) * P], 1.0)
                    if full_t:
                        nc.sync.dma_start(
                            out=wa[:, 0 : full_t * P].rearrange(
                                "p (t d) -> p t d", d=D
                            ),
                            in_=w_ext[r0 : r0 + full_t * P, :].rearrange(
                                "(t p) d -> p t d", p=P
                            ),
                        )
                    if rem:
                        nc.sync.dma_start(
                            out=wa[0:rem, full_t * P : full_t * P + D],
                            in_=w_ext[r0 + full_t * P : r0 + rows, :],
                        )
                if k >= 2:
                    kk = k - 2
                    wa, nt, r0, rows, full_t, rem = was[kk]
                    for t in range(nt):
                        gi = kk * wbatch + t
                        scr = smallp.tile([P, P], f32, tag="ttr_scr", name="ttr_scr")
                        nc.vector.scalar_tensor_tensor(
                            out=scr[:], in0=wa[:, t * P : (t + 1) * P], scalar=1.0,
                            in1=wa[:, t * P : (t + 1) * P], op0=OP.mult, op1=OP.mult,
                            accum_out=ssqW[:, gi : gi + 1],
                        )
                    rsqrt(ssqW[:, kk * wbatch : kk * wbatch + nt],
                          invW[:, kk * wbatch : kk * wbatch + nt], nt)
                    wscb = wscp.tile([P, wbatch * P], bf16, tag="wsc", name="wscb")
                    for t in range(nt):
                        gi = kk * wbatch + t
                        nc.vector.tensor_scalar(
                            out=wscb[:, t * P : (t + 1) * P],
                            in0=wa[:, t * P : (t + 1) * P],
                            scalar1=invW[:, gi : gi + 1], scalar2=None, op0=OP.mult,
                        )
                    if kk < pe_tp_batches:
                        # TensorE transpose path: immune to the collectives
                        # entry barrier that serializes xbar transposes, so
                        # the first chunks can start the main loop early.
                        for t in range(nt):
                            gi = kk * wbatch + t
                            tpp = psmm.tile([P, P], bf16, tag="pm", name="tp_w")
                            nc.tensor.transpose(
                                out=tpp[:],
                                in_=wscb[:, t * P : (t + 1) * P],
                                identity=ident[:],
                            )
                            nc.vector.tensor_copy(
                                out=wT[:, gi * P : (gi + 1) * P], in_=tpp[:]
                            )
                    else:
                        if full_t:
                            nc.gpsimd.dma_start(
                                out=wbf_d[r0 : r0 + full_t * P, :].rearrange(
                                    "(t p) d -> p t d", p=P
                                ),
                                in_=wscb[:, 0 : full_t * P].rearrange(
                                    "p (t d) -> p t d", d=D
                                ),
                            )
                        if rem:
                            nc.gpsimd.dma_start(
                                out=wbf_d[r0 + full_t * P : r0 + rows, :],
                                in_=wscb[0:rem, full_t * P : full_t * P + D],
                            )
                        bend = min(wbatch * P, scol - r0)
                        nc.sync.dma_start_transpose(
                            out=wT[:, r0 : r0 + bend], in_=wbf_d[r0 : r0 + bend, :]
                        )
            for b in range(nb):
                scr = smallp.tile([P, P], f32, tag="ttr_scr", name="ttr_scr")
                nc.vector.scalar_tensor_tensor(
                    out=scr[:], in0=wlabs[b][:], scalar=1.0, in1=wlabs[b][:],
                    op0=OP.mult, op1=OP.mult, accum_out=ssqL[:, b : b + 1],
                )
            rsqrt(ssqL[:, 0:nb], invL[:, 0:nb], nb)
            for b in range(nb):
                scr = smallp.tile([P, P], f32, tag="ttr_scr", name="ttr_scr")
                nc.vector.scalar_tensor_tensor(
                    out=scr[:], in0=wlabs[b][:], scalar=invL[:, b : b + 1],
                    in1=fnorm[:, b * P : (b + 1) * P], op0=OP.mult, op1=OP.mult,
                    accum_out=dots[:, b : b + 1],
                )
            scrl = smallp.tile([P, nb], f32, tag="scrm", name="scrl")
            nc.vector.tensor_tensor(out=scrl[:], in0=dots[:], in1=mask[:], op=OP.mult)
            nc.vector.tensor_scalar(
                out=ccin[:, 2 * nb : 3 * nb], in0=scrl[:], scalar1=S, scalar2=None,
                op0=OP.mult,
            )

            if scol > cs:
                # zero the pad columns so they contribute ~exp(-30) ~ 0 to Z
                nc.vector.memset(wT[:, cs:scol], 0.0)

            # s vector = sum over classes of w_hat (free-axis reduce of wT)
            nc.vector.tensor_reduce(
                out=s32[:], in_=wT[:, 0:cs], axis=mybir.AxisListType.X, op=OP.add
            )
            nc.vector.tensor_copy(out=wT[:, scol : scol + 1], in_=s32[:])

            # ---- main loop: matmul chunks + fused exp-accumulate ------------
            # chunk-outer so the loop starts as soon as the first weight
            # batches are transposed (wT fills left to right)
            for cc in range(nchunk):
                c0 = cc * chunk
                cw = min(chunk, scol - c0)
                for b in range(nb):
                    lhs = fT[:, b * P : (b + 1) * P]
                    pm = psmm.tile([P, chunk], f32, tag="pm", name="pm")
                    for sgi in range(math.ceil(cw / 512)):
                        n0 = sgi * 512
                        nn = min(512, cw - n0)
                        nc.tensor.matmul(
                            out=pm[:, n0 : n0 + nn],
                            lhsT=lhs,
                            rhs=wT[:, c0 + n0 : c0 + n0 + nn],
                            start=True,
                            stop=True,
                        )
                    nc.scalar.activation(
                        out=pm[:, 0:cw],
                        in_=pm[:, 0:cw],
                        func=FT.Exp,
                        bias=bias_m30[:, 0:1],
                        scale=S,
                        accum_out=ZP[:, b * nchunk + cc : b * nchunk + cc + 1],
                    )

            # T columns: one N=1 matmul per B tile against the s column
            for b in range(nb):
                ts = psmm.tile([P, 16], f32, tag="pm", name="ts")
                nc.tensor.matmul(
                    out=ts[:, 0:1],
                    lhsT=fT[:, b * P : (b + 1) * P],
                    rhs=wT[:, scol : scol + 1],
                    start=True,
                    stop=True,
                )
                nc.vector.tensor_scalar(
                    out=ccin[:, nb + b : nb + b + 1], in0=ts[:, 0:1], scalar1=S,
                    scalar2=None, op0=OP.mult,
                )

            # Z columns: reduce the per-chunk partials
            nc.vector.tensor_reduce(
                out=ccin[:, 0:nb],
                in_=ZP[:].rearrange("p (b c) -> p b c", c=nchunk),
                axis=mybir.AxisListType.X,
                op=OP.add,
            )

            # ---- AllGather of (Z | T | L) + local rank-reduce ---------------
            # (AllGather floor ~5us vs ~56us for an 8-rank ring AllReduce)
            ccin_d = dramp.tile([P, 3 * nb], f32, name="ccin_d")
            ccout_d = dramp.tile([num_cores * P, 3 * nb], f32, name="ccout_d")
            nc.sync.dma_start(out=ccin_d[:], in_=ccin[:])
            nc.gpsimd.collective_compute(
                "AllGather",
                OP.bypass,
                replica_groups=[list(range(num_cores))],
                ins=[ccin_d.opt()],
                outs=[ccout_d.opt()],
            )
            R8 = persist.tile([P, 3 * nb * num_cores], f32, name="R8")
            # one contiguous DMA per rank (a strided gather here shatters into
            # 4-byte descriptors and costs 60+us)
            w3 = 3 * nb
            for r in range(num_cores):
                nc.sync.dma_start(
                    out=R8[:, r * w3 : (r + 1) * w3],
                    in_=ccout_d[r * P : (r + 1) * P, :],
                )
            nc.vector.tensor_reduce(
                out=R[:],
                in_=R8[:].rearrange("p (r c) -> p c r", r=num_cores),
                axis=mybir.AxisListType.X,
                op=OP.add,
            )

            # ---- final loss -------------------------------------------------
            Zg = R[:, 0:nb]
            Tg = R[:, nb : 2 * nb]
            Lg = R[:, 2 * nb : 3 * nb]
            expL = smallp.tile([P, nb], f32, tag="fin", name="expL")
            zadj = smallp.tile([P, nb], f32, tag="fin2", name="zadj")
            lnz = smallp.tile([P, nb], f32, tag="fin3", name="lnz")
            t1 = smallp.tile([P, nb], f32, tag="fin4", name="t1")
            losscol = smallp.tile([P, 1], f32, tag="fin6", name="losscol")
            outsb = smallp.tile([1, 1], f32, tag="fin8", name="outsb")

            nc.scalar.activation(
                out=expL[:], in_=Lg, func=FT.Exp, bias=bias_m30[:, 0:1], scale=1.0
            )
            nc.vector.scalar_tensor_tensor(
                out=zadj[:], in0=expL[:], scalar=kappa_m1, in1=Zg,
                op0=OP.mult, op1=OP.add,
            )
            nc.scalar.activation(out=lnz[:], in_=zadj[:], func=FT.Ln)
            nc.vector.scalar_tensor_tensor(
                out=t1[:], in0=Lg, scalar=-(1.0 - EPS), in1=lnz[:],
                op0=OP.mult, op1=OP.add,
            )
            nc.vector.scalar_tensor_tensor(
                out=t1[:], in0=Tg, scalar=-EPS / c_total, in1=t1[:],
                op0=OP.mult, op1=OP.add,
            )
            nc.vector.tensor_reduce(
                out=losscol[:], in_=t1[:], axis=mybir.AxisListType.X, op=OP.add
            )
            ones_ap = nc.const_aps.aps[(f32, 1.0)]
            pfin = psmm.tile([P, 16], f32, tag="pm", name="pfin")
            nc.tensor.matmul(
                out=pfin[0:1, 0:1], lhsT=losscol[:, 0:1], rhs=ones_ap[:, 0:1],
                start=True, stop=True,
            )
            nc.vector.tensor_scalar(
                out=outsb[:], in0=pfin[0:1, 0:1], scalar1=1.0 / b_sz,
                scalar2=const, op0=OP.mult, op1=OP.add,
            )
            nc.sync.dma_start(out=out_ext[:, :], in_=outsb[:])

    nc.compile()
    return nc


def make_in_maps(features, labels, weight, num_cores=NCORES, b_sz=B, cs=CS):
    nb = b_sz // P
    f = np.ascontiguousarray(np.asarray(features, dtype=np.float32))
    lab = np.ascontiguousarray(np.asarray(labels).astype(np.int32).reshape(P, nb))
    w = np.asarray(weight, dtype=np.float32)
    in_maps = []
    for k in range(num_cores):
        in_maps.append(
            {
                "f": f,
                "w": np.ascontiguousarray(w[k * cs : (k + 1) * cs]),
                "lab": lab,
                "coff": np.full((P, 1), k * cs, dtype=np.float32),
                "id32": np.eye(P, dtype=np.float32),
            }
        )
    return in_maps


_NC_CACHE = {}


def kernel(features, labels, weight):
    from concourse.bass_utils import run_bass_kernel_spmd

    if "nc" not in _NC_CACHE:
        _NC_CACHE["nc"] = build_graph()
    nc = _NC_CACHE["nc"]
    in_maps = make_in_maps(features, labels, weight)
    res = run_bass_kernel_spmd(nc, in_maps, core_ids=list(range(NCORES)))
    return np.float32(res.results[0]["out"][0, 0])
